# revision 18
# baseline (speedup 1.0000x reference)
"""Causal self-attention (RoPE) Trainium2 Bass kernel, 8-core SPMD.

Sharding: core c -> batch c//2, head-group c%2 (8 of 16 heads).
Per core: q/k/v projections column-sharded over heads, block-causal attention
for its 8 heads, out-projection row-sharded; the host sums the two partial
outputs per batch and adds bo.

All matmuls run in 64x128 row-tiled PE mode (tile_size (64,128), zero mode
switches). Scores for a head PAIR run as two concurrent row-tiles (T0 rows
0-63 = even head, T8 rows 64-127 = odd head; measured 109ns/MM vs 216 for
128-mode). Projections / pv / out-proj run "crosswise": two psum
accumulators in flight, T0 takes the low contraction half of one while T8
takes the high half of the other, then they swap (measured parity with
128-mode). Attention per (pair, 512-q chunk J) walks kv tiles of 128;
diagonal tiles (i-4J=r>=0) are narrowed to q columns [128r, 512) and the
remaining 128x128 triangle is masked by one bf16 multiply after exp.
Softmax denominators come from a ones-column in v_ext (row 64 of yt);
reciprocals are batched per pair into one [8,512] DVE reciprocal (cost is
free-dim proportional: 3.34us regardless of partition count); pair 3
normalizes per-chunk so the out-projection chunks can overlap its
attention. QK projection for pair m+1 and the v projection tail drip into
the ACT-paced attention stream one 4-MM granule at a time via a fill
queue; out-proj granules drip into pair 3.
"""
import sys

sys.path.insert(0, "/opt/trn_rl_repo")

import math
from contextlib import ExitStack

import ml_dtypes
import numpy as np

import concourse.bass as bass
import concourse.tile as tile
from concourse import bacc, mybir
from concourse.bass_utils import run_bass_kernel_spmd

F32 = mybir.dt.float32
BF16 = mybir.dt.bfloat16
AF = mybir.ActivationFunctionType

N_CORES = 8
B, T, D = 4, 2048, 1024
H, HD = 16, 64          # total heads, head dim
HC = 8                  # heads per core
DC = HC * HD            # 512 sharded projection dims per core
BASE = 10000
NT = T // 128           # 16 t-tiles
NM = DC // 128          # 4 m-tiles of q/k (2 heads each)
NK = D // 128           # 8 contraction tiles of D
NQ = T // 512           # 4 q-chunks
VW = HC * (HD + 1)      # 520: v_ext width (64 dims + ones col per head)


def _build_program():
    nc = bacc.Bacc("TRN2", target_bir_lowering=False, debug=False,
                   num_devices=N_CORES)

    def din(name, shape, dt):
        return nc.dram_tensor(name, shape, dt, kind="ExternalInput").ap()

    xT = din("xT", [D, T], BF16)              # x[b].T
    wqT = din("wqT", [D, DC], BF16)           # (Wq/8)[rows].T
    wkT = din("wkT", [D, DC], BF16)
    wvT = din("wvT", [D, DC], BF16)
    woT = din("woT", [DC, D], BF16)           # Wo[:, rows].T
    bqk_cols = din("bqk_cols", [128, 8], F32)  # q bias m-cols 0-3, k bias 4-7
    bv_row = din("bv_row", [1, DC], F32)
    ones_col = din("ones_col", [128, 8], F32)  # ones block for v_ext columns
    cosS = din("cosS", [128, T], BF16)        # 2-head-stacked cos table
    sinS = din("sinS", [128, T], BF16)        # sign-folded sin table
    triS = din("triS", [128, 256], BF16)      # [tri|tri] causal KEEP triangle
    outT = nc.dram_tensor("outT", [D, T], BF16,
                          kind="ExternalOutput").ap()

    # round-robin DMA issue over engine queues to parallelize HBM fetch
    dq = []

    def dma(dst, src):
        q = dq[0]
        dq.append(dq.pop(0))
        q.dma_start(dst, src)

    mm = nc.tensor.matmul

    with tile.TileContext(nc) as tc, ExitStack() as top:
        dq.extend([nc.sync, nc.scalar])
        p_const = top.enter_context(tc.tile_pool(name="const", bufs=1))
        bqk_t = p_const.tile([128, 8], F32, name="bqk_t")
        dma(bqk_t[:], bqk_cols[:])
        bv_t = p_const.tile([1, DC], F32, name="bv_t")
        dma(bv_t[:], bv_row[:])
        bvb = p_const.tile([128, DC], F32, name="bvb")
        nc.gpsimd.partition_broadcast(bvb[:], bv_t[:])
        ones_col_t = p_const.tile([128, 8], F32, name="ones_col_t")
        dma(ones_col_t[:], ones_col[:])

        p_yn = top.enter_context(tc.tile_pool(name="yn", bufs=1))
        yn = [p_yn.tile([128, T], BF16, name=f"yn{m}") for m in range(NM)]
        p_v = top.enter_context(tc.tile_pool(name="pv_ext", bufs=1))
        v_ext = [p_v.tile([128, VW], BF16, name=f"vext{tt}")
                 for tt in range(NT)]
        p_wo = top.enter_context(tc.tile_pool(name="po_w", bufs=1))

        with tc.tile_pool(name="pw", bufs=1) as p_w, \
             tc.tile_pool(name="px", bufs=1) as p_x, \
             tc.tile_pool(name="ptmp", bufs=3) as p_tmp, \
             tc.tile_pool(name="pqk", bufs=1) as p_qk, \
             tc.tile_pool(name="ppt", bufs=4) as p_pt, \
             tc.tile_pool(name="pysb", bufs=8) as p_ysb, \
             tc.tile_pool(name="pz", bufs=2) as p_z, \
             tc.tile_pool(name="pst", bufs=3) as p_st, \
             tc.tile_pool(name="ps_acc", bufs=2, space="PSUM") as ps_acc, \
             tc.tile_pool(name="ps_sb", bufs=2, space="PSUM") as ps_sb, \
             tc.tile_pool(name="ps_yt", bufs=2, space="PSUM") as ps_yt:
            # ---- input DMAs, first-needed-first ----
            wq, wk, wv = [], [], []
            xs = [[None] * NQ for _ in range(NK)]
            for k in range(NK):
                xt = p_x.tile([128, 512], BF16, name=f"xs{k}q0")
                dma(xt[:], xT[bass.ts(k, 128), 0:512])
                xs[k][0] = xt
                wt = p_w.tile([128, DC], BF16, name=f"wq{k}")
                dma(wt[:], wqT[bass.ts(k, 128), :])
                wq.append(wt)
            for k in range(NK):
                wt = p_w.tile([128, DC], BF16, name=f"wk{k}")
                dma(wt[:], wkT[bass.ts(k, 128), :])
                wk.append(wt)
            cos_t = p_const.tile([128, T], BF16, name="cos_t")
            dma(cos_t[:], cosS[:])
            sin_t = p_const.tile([128, T], BF16, name="sin_t")
            dma(sin_t[:], sinS[:])
            tri_t = p_const.tile([128, 256], BF16, name="tri_t")
            dma(tri_t[:], triS[:])
            for k in range(NK):
                wt = p_w.tile([128, DC], BF16, name=f"wv{k}")
                dma(wt[:], wvT[bass.ts(k, 128), :])
                wv.append(wt)
            for qtr in range(1, NQ):
                for k in range(NK):
                    xt = p_x.tile([128, 512], BF16, name=f"xs{k}q{qtr}")
                    dma(xt[:], xT[bass.ts(k, 128), bass.ts(qtr, 512)])
                    xs[k][qtr] = xt
            wo = []
            for k in range(NM):
                wt = p_wo.tile([128, D], BF16, name=f"wo{k}")
                nc.sync.dma_start(wt[:], woT[bass.ts(k, 128), :])
                wo.append(wt)

            qTm = [p_qk.tile([128, T], BF16, name=f"qTm{m}")
                   for m in range(NM)]
            kTm = [p_qk.tile([128, T], BF16, name=f"kTm{m}")
                   for m in range(NM)]

            # ---- crosswise generators (each yield = one 4-MM granule) ----
            def gen_v(tt):
                qtr, tl = divmod(tt, 4)
                acc = ps_acc.tile([128, DC], F32, name="acc_v", tag="acc",
                                  bufs=2)
                for k in range(NK):
                    mm(acc[:], xs[k][qtr][:, bass.ts(tl, 128)], wv[k][:],
                       start=(k == 0), stop=(k == NK - 1))
                    if k % 2 == 1:
                        yield
                v3 = v_ext[tt][:].rearrange("p (h w) -> p h w", w=HD + 1)
                nc.gpsimd.tensor_copy(
                    v3[:, :, HD:HD + 1],
                    ones_col_t[:].rearrange("p (h w) -> p h w", w=1))
                nc.vector.tensor_add(
                    v3[:, :, 0:HD],
                    acc[:].rearrange("p (h w) -> p h w", w=HD),
                    bvb[:].rearrange("p (h w) -> p h w", w=HD))

            def rope_emit(acc, dest, qtr, bcol):
                hs = bass.ts(qtr, 512)
                qb = p_tmp.tile([128, 512], BF16, name="rope_qb",
                                tag="rope_qb", bufs=3)
                nc.vector.tensor_scalar_add(
                    qb[:], acc[:], bqk_t[:, bcol:bcol + 1])
                shuf = p_tmp.tile([128, 512], BF16, name="rope_shuf",
                                  tag="rope_shuf", bufs=3)
                for (dst, src) in ((0, 32), (32, 0), (64, 96), (96, 64)):
                    nc.gpsimd.dma_start(shuf[dst:dst + 32, :],
                                        qb[src:src + 32, :])
                t1 = p_tmp.tile([128, 512], BF16, name="rope_t1",
                                tag="rope_t1", bufs=3)
                nc.vector.tensor_mul(t1[:], qb[:], cos_t[:, hs])
                nc.vector.tensor_mul(shuf[:], shuf[:], sin_t[:, hs])
                nc.vector.tensor_add(dest[:, hs], t1[:], shuf[:])

            def gen_qk_pair(m, qtr):
                mc = bass.ts(m, 128)
                accQ = ps_acc.tile([128, 512], F32, name="accQ", tag="acc",
                                   bufs=2)
                for k in range(NK):
                    mm(accQ[:], wq[k][:, mc], xs[k][qtr][:],
                       start=(k == 0), stop=(k == NK - 1))
                    if k % 2 == 1:
                        yield
                rope_emit(accQ, qTm[m], qtr, m)
                accK = ps_acc.tile([128, 512], F32, name="accK", tag="acc",
                                   bufs=2)
                for k in range(NK):
                    mm(accK[:], wk[k][:, mc], xs[k][qtr][:],
                       start=(k == 0), stop=(k == NK - 1))
                    if k % 2 == 1:
                        yield
                rope_emit(accK, kTm[m], qtr, 4 + m)

            def gen_out_pair(j2, J):
                qs = bass.ts(J, 512)
                for M in (2 * j2, 2 * j2 + 1):
                    acc = ps_acc.tile([128, 512], F32, name="acc_o",
                                      tag="acc", bufs=2)
                    for mk in range(NM):
                        mm(acc[:], wo[mk][:, bass.ts(M, 128)],
                           yn[mk][:, qs], start=(mk == 0),
                           stop=(mk == NM - 1))
                        if mk % 2 == 1:
                            yield
                    st = p_st.tile([128, 512], BF16, name="out_st",
                                   tag="st", bufs=3)
                    nc.vector.tensor_copy(st[:], acc[:])
                    if M % 2 == 0:
                        nc.sync.dma_start(outT[bass.ts(M, 128), qs], st[:])
                    else:
                        nc.gpsimd.dma_start(outT[bass.ts(M, 128), qs],
                                            st[:])

            # ---- fill queues ----
            # fillq items: (need_key, generator); need_key = (m, J) means
            # "must be fully emitted before attention chunk A(m, J)".
            fillq = []
            outq = []    # out-proj generators (pair-3 / tail pops)
            dveq = []    # deferred normalization actions

            def pop_fill(n):
                for _ in range(n):
                    while fillq:
                        try:
                            next(fillq[0][1])
                            break
                        except StopIteration:
                            fillq.pop(0)
                    else:
                        while outq:
                            try:
                                next(outq[0])
                                break
                            except StopIteration:
                                outq.pop(0)
                        else:
                            return

            def drain_until(key):
                while fillq and fillq[0][0] <= key:
                    try:
                        next(fillq[0][1])
                    except StopIteration:
                        fillq.pop(0)

            def norm_one(m, hh, J, ysb, zrec_box):
                def act():
                    zr1 = p_z.tile([1, 512], F32, name="zr1", tag="zr1",
                                   bufs=2)
                    r = zrec_box[1][(m, hh, J)]
                    nc.sync.dma_start(zr1[:], zrec_box[0][r:r + 1, :])
                    zb = p_z.tile([64, 512], F32, name="zb", tag="zb",
                                  bufs=2)
                    nc.gpsimd.partition_broadcast(zb[:], zr1[:])
                    nc.vector.tensor_mul(
                        yn[m][64 * hh:64 * hh + 64, bass.ts(J, 512)],
                        ysb[0:64, :], zb[:])
                return act

            def norm_one3(m, hh, J, ysb, zrec_box):
                def act():
                    zr1 = p_z.tile([1, 512], F32, name="zr1", tag="zr1",
                                   bufs=2)
                    nc.sync.dma_start(zr1[:],
                                      zrec_box[0][8 * hh:8 * hh + 8, :])
                    zb = p_z.tile([64, 512], F32, name="zb", tag="zb",
                                  bufs=2)
                    nc.gpsimd.partition_broadcast(zb[:], zr1[:])
                    nc.vector.tensor_mul(
                        yn[m][64 * hh:64 * hh + 64, bass.ts(J, 512)],
                        ysb[0:64, :], zb[:])
                return act

            # ---- attention, pair-major ----
            # warmup: v tiles 0..3 + qk(0,0) emitted dense up front
            for _ in gen_qk_pair(0, 0):
                pass
            for tt in range(NT):
                for _ in gen_v(tt):
                    pass
            for _ in gen_qk_pair(0, 1):
                pass
            for qtr in range(2, NQ):
                fillq.append(((0, qtr), gen_qk_pair(0, qtr)))
            for mn in range(1, NM):
                for qtr in range(NQ):
                    kk = max(1, mn * NQ + qtr - 2)
                    fillq.append(((kk // NQ, kk % NQ),
                                  gen_qk_pair(mn, qtr)))

            for m in range(NM):
                last_pair = m == NM - 1
                if not last_pair:
                    zden_t = p_z.tile([8, 512], F32, name="zden",
                                      tag="zden", bufs=2)
                    pair_rows = []
                unit = 0
                for J in range(NQ):
                    drain_until((m, J))
                    ntiles = 4 * J + 4
                    yt0 = ps_yt.tile([65, 512], F32, name="yt0", tag="yt",
                                     bufs=2)
                    yt1 = ps_yt.tile([65, 512], F32, name="yt1", tag="yt",
                                     bufs=2)
                    pend = []

                    def emit_pv(ent, yt0=None, yt1=None, m=m, ntiles=None):
                        # 128-mode pv: full-kv contraction, one MM per head
                        i, pt_t, qlo = ent
                        v3 = v_ext[i][:].rearrange("p (h w) -> p h w",
                                                   w=HD + 1)
                        e0 = i == 0
                        el = i == ntiles - 1
                        mm(yt0[:, qlo:512], v3[:, 2 * m, :],
                           pt_t[:, qlo:512], start=e0, stop=el)
                        mm(yt1[:, qlo:512], v3[:, 2 * m + 1, :],
                           pt_t[:, 512 + qlo:1024], start=e0, stop=el)

                    # two score tiles per mode phase (64-mode pair bursts,
                    # then a 128-mode burst of pv + projection granules)
                    for i2 in range(0, ntiles, 2):
                        for i in (i2, i2 + 1):
                            r = i - 4 * J
                            qlo = 128 * r if r >= 0 else 0
                            lo, hi = 512 * J + qlo, 512 * J + 512
                            sb = ps_sb.tile([128, 1024], F32, name="sb",
                                            tag="sb", bufs=2)
                            mm(sb[:, qlo:512],
                               kTm[m][0:64, bass.ts(i, 128)],
                               qTm[m][0:64, lo:hi], start=True, stop=True)
                            mm(sb[:, 512 + qlo:1024],
                               kTm[m][64:128, bass.ts(i, 128)],
                               qTm[m][64:128, lo:hi], start=True, stop=True)
                            pt_t = p_pt.tile([128, 1024], BF16, name="pt",
                                             tag="pt", bufs=4)
                            sb3 = sb[:].rearrange("p (g w) -> p g w",
                                                  g=2)[:, :, qlo:512]
                            pt3 = pt_t[:].rearrange("p (g w) -> p g w",
                                                    g=2)[:, :, qlo:512]
                            nc.scalar.activation(pt3, sb3, AF.Exp,
                                                 scale=1.0)
                            if r >= 0:
                                ptm = pt_t[:].rearrange(
                                    "p (g w) -> p g w",
                                    g=2)[:, :, qlo:qlo + 128]
                                nc.vector.tensor_mul(
                                    ptm, ptm,
                                    tri_t[:].rearrange("p (g w) -> p g w",
                                                       g=2))
                            pend.append((i, pt_t, qlo))
                        thresh = 4 if J == 0 else 2
                        for _ in range(2):
                            if len(pend) > thresh:
                                emit_pv(pend.pop(0), yt0=yt0, yt1=yt1,
                                        ntiles=ntiles)
                        # drips: norm actions + projection granules
                        for _ in range(3 if last_pair else 2):
                            if dveq:
                                dveq.pop(0)()
                                if last_pair and dveq:
                                    dveq.pop(0)()
                        tail_du = (J == NQ - 1 and i2 >= ntiles - 4)
                        if not tail_du:
                            pop_fill(3 if 1 <= m <= 2 else 2)
                        unit += 2
                    for ent in pend:
                        emit_pv(ent, yt0=yt0, yt1=yt1, ntiles=ntiles)
                    # pair/chunk epilogue: evacuate yt, gather denominators
                    ysbs = []
                    for hh, yt in ((0, yt0), (1, yt1)):
                        ysb = p_ysb.tile([65, 512], F32, name="ysb",
                                         tag="ysb", bufs=8)
                        nc.vector.tensor_copy(ysb[:], yt[:])
                        ysbs.append((hh, ysb))
                    if not last_pair:
                        for hh, ysb in ysbs:
                            nc.sync.dma_start(
                                zden_t[2 * J + hh:2 * J + hh + 1, :],
                                ysb[64:65, :])
                            pair_rows.append((hh, J, ysb))
                    else:
                        # immediate per-chunk normalize for the last pair;
                        # [2,512] denom rows DMA-reshaped to [16,64] so the
                        # DVE reciprocal costs ~0.55us instead of 3.34us
                        zdT = p_z.tile([16, 64], F32, name="zdT",
                                       tag="zdT", bufs=2)
                        for hh, ysb in ysbs:
                            nc.sync.dma_start(zdT[8 * hh:8 * hh + 8, :],
                                              ysb[64:65, :])
                        zrec_box = [None, {}]

                        def recip3(zdT=zdT, zrec_box=zrec_box):
                            zrT = p_z.tile([16, 64], F32, name="zrT",
                                           tag="zrT", bufs=2)
                            nc.vector.reciprocal(zrT[:], zdT[:])
                            zrec_box[0] = zrT

                        dveq.append(recip3)
                        for hh, ysb in ysbs:
                            dveq.append(norm_one3(m, hh, J, ysb, zrec_box))
                        outq.append(gen_out_pair(0, J))
                        outq.append(gen_out_pair(1, J))
                        outq.append(gen_out_pair(2, J))
                        outq.append(gen_out_pair(3, J))
                if not last_pair:
                    # batched pair normalization, dripped into pair m+1
                    zrec_box = [None, {}]

                    def recip8(zden_t=zden_t, zrec_box=zrec_box):
                        zrec = p_z.tile([8, 512], F32, name="zrec",
                                        tag="zrec", bufs=2)
                        nc.vector.reciprocal(zrec[:], zden_t[:])
                        zrec_box[0] = zrec

                    dveq.append(recip8)
                    for hh, J, ysb in pair_rows:
                        zrec_box[1][(m, hh, J)] = 2 * J + hh
                        dveq.append(norm_one(m, hh, J, ysb, zrec_box))
            # ---- drain tail ----
            while dveq:
                dveq.pop(0)()
            pop_fill(10 ** 6)
            while outq:
                try:
                    next(outq[0])
                except StopIteration:
                    outq.pop(0)

    nc.compile()
    return nc


_NC_CACHE = None


def _get_program():
    global _NC_CACHE
    if _NC_CACHE is None:
        _NC_CACHE = _build_program()
    return _NC_CACHE


def _host_inputs(x, Wq, bq, Wk, bk, Wv, bv, Wo, bo):
    scale = 1.0 / math.sqrt(HD)
    Wq_s = (np.asarray(Wq, dtype=np.float32) * scale).astype(np.float32)
    bq_s = (np.asarray(bq, dtype=np.float32) * scale).astype(np.float32)
    x = np.asarray(x, dtype=np.float32)
    Wk = np.asarray(Wk, dtype=np.float32)
    Wv = np.asarray(Wv, dtype=np.float32)
    Wo = np.asarray(Wo, dtype=np.float32)
    bk = np.asarray(bk, dtype=np.float32)
    bv = np.asarray(bv, dtype=np.float32)

    # rope tables, 2-head-stacked [128, T]
    j = np.arange(HD // 2, dtype=np.float64)
    theta = BASE ** (-2.0 * j / HD)                      # [32]
    pos = np.arange(1, T + 1, dtype=np.float64)          # [T]
    ang = pos[None, :] * theta[:, None]                  # [32, T]
    cos32 = np.cos(ang)
    sin32 = np.sin(ang)
    cos64 = np.concatenate([cos32, cos32], axis=0)       # [64, T]
    sin64 = np.concatenate([-sin32, sin32], axis=0)      # sign-folded
    cosS = np.concatenate([cos64, cos64], axis=0).astype(np.float32)
    sinS = np.concatenate([sin64, sin64], axis=0).astype(np.float32)

    p = np.arange(128)
    f = np.arange(128)
    tri = (p[:, None] <= f[None, :]).astype(np.float32)  # [128,128] keep
    triS = np.concatenate([tri, tri], axis=1)            # [128, 256]

    in_maps = []
    for c in range(N_CORES):
        b, g = c // 2, c % 2
        rows = slice(DC * g, DC * (g + 1))
        bqk = np.zeros((128, 8), dtype=np.float32)
        for m in range(NM):
            bqk[:, m] = bq_s[rows][128 * m:128 * (m + 1)]
            bqk[:, 4 + m] = bk[rows][128 * m:128 * (m + 1)]
        bf = ml_dtypes.bfloat16
        in_maps.append({
            "xT": np.ascontiguousarray(x[b].T).astype(bf),
            "wqT": np.ascontiguousarray(Wq_s[rows].T).astype(bf),
            "wkT": np.ascontiguousarray(Wk[rows].T).astype(bf),
            "wvT": np.ascontiguousarray(Wv[rows].T).astype(bf),
            "woT": np.ascontiguousarray(Wo[:, rows].T).astype(bf),
            "bqk_cols": bqk,
            "bv_row": bv[rows].reshape(1, DC),
            "ones_col": np.ones((128, 8), dtype=np.float32),
            "cosS": cosS.astype(bf),
            "sinS": sinS.astype(bf),
            "triS": triS.astype(bf),
        })
    return in_maps


def kernel(x, Wq, bq, Wk, bk, Wv, bv, Wo, bo, _trace=False, _tmpdir=None):
    nc = _get_program()
    in_maps = _host_inputs(x, Wq, bq, Wk, bk, Wv, bv, Wo, bo)
    res = run_bass_kernel_spmd(nc, in_maps, list(range(N_CORES)),
                               trace=_trace, tmpdir=_tmpdir)
    kernel.last_exec_time_ns = res.exec_time_ns
    bo = np.asarray(bo, dtype=np.float32)
    out = np.zeros((B, T, D), dtype=np.float32)
    for b in range(B):
        acc = res.results[2 * b]["outT"].astype(np.float32) + \
            res.results[2 * b + 1]["outT"].astype(np.float32)
        out[b] = acc.T + bo[None, :]
    return out
